# revision 1
# baseline (speedup 1.0000x reference)
import sys, os
for _p in ("/opt/trn_rl_repo", "/root/.axon_site/_ro/trn_rl_repo"):
    if os.path.isdir(_p) and _p not in sys.path:
        sys.path.insert(0, _p)
import numpy as np
import ml_dtypes

BF16 = ml_dtypes.bfloat16

NUM_HEADS = 8
HEAD_DIM = 32
COORDS_DIM = 3
NUM_W_PER_DIST = 8
BLOCK_SIZE = 256
N = 65536
NCORES = 8
NB_PER_CORE = (N // BLOCK_SIZE) // NCORES  # 32 blocks/core
B = BLOCK_SIZE
H = NUM_HEADS
D = HEAD_DIM

_CACHE = {}


def _build_nc():
    import concourse.bass as bass
    import concourse.mybir as mybir

    nc = bass.Bass()
    bf = mybir.dt.bfloat16
    f32 = mybir.dt.float32
    Exp = mybir.ActivationFunctionType.Exp

    NBLK = NB_PER_CORE
    HB = H * B
    W = H * 2 * 33
    qa_d = nc.declare_dram_parameter("qa", [NBLK, 36, HB], bf, isOutput=False)
    ka_d = nc.declare_dram_parameter("ka", [NBLK, 36, HB], bf, isOutput=False)
    va_d = nc.declare_dram_parameter("va", [NBLK, 128, W], bf, isOutput=False)
    out_d = nc.declare_dram_parameter("onum", [NBLK, 33, HB], f32, isOutput=True)

    # cumulative-count helpers (emission order == execution order per engine)
    def act_after(b, h):        # exps completed through (b, h)
        return 8 * b + h + 1

    def dve_after(b, h):        # copies completed through (b, h)
        return 8 * b + h + 1

    def pe_after_scores(b, h):
        return 32 * b + (2 if h == 0 else 4 * h + 2)

    def pe_after_av(b, h):      # h in 0..7
        return 32 * b + (4 * (h + 1) + 2 if h < 7 else 32)

    with (
        nc.sbuf_tensor([36, 2 * HB], bf) as qa_t,
        nc.sbuf_tensor([36, 2 * HB], bf) as ka_t,
        nc.sbuf_tensor([128, 2 * W], bf) as va_t,
        nc.sbuf_tensor([128, 2 * 2 * B], bf) as es_t,
        nc.sbuf_tensor([33, 2 * HB], f32) as avsb_t,
        nc.psum_tensor([128, 2 * 2 * B], f32) as sc_p,   # 2 slots x 1 bank
        nc.psum_tensor([33, 2 * 2 * B], f32) as av_p,    # 2 slots, bank-strided
        nc.semaphore("dmain") as dmain,
        nc.semaphore("dmaout") as dmaout,
        nc.semaphore("pe") as pe,
        nc.semaphore("act") as act,
        nc.semaphore("dve") as dve,
        nc.Block() as block,
    ):
        @block.sync
        def _(sync):
            def load(b):
                s = b % 2
                if b >= 2:
                    sync.wait_ge(pe, 32 * (b - 1))
                sync.dma_start(qa_t[:, s * HB:(s + 1) * HB], qa_d[b]).then_inc(dmain, 16)
                sync.dma_start(ka_t[:, s * HB:(s + 1) * HB], ka_d[b]).then_inc(dmain, 16)
                sync.dma_start(va_t[:, s * W:(s + 1) * W], va_d[b]).then_inc(dmain, 16)

            load(0)
            load(1)
            for b in range(NBLK):
                if b + 2 < NBLK:
                    load(b + 2)
                s = b % 2
                sync.wait_ge(dve, 8 * (b + 1))
                sync.dma_start(out_d[b], avsb_t[:, s * HB:(s + 1) * HB]).then_inc(dmaout, 16)

        @block.tensor
        def _(tensor):
            def emit_av(b, h):
                s = b % 2
                p = h % 2
                tensor.wait_ge(act, act_after(b, h))
                v = 8 * b + h - 1
                if v > 0:
                    tensor.wait_ge(dve, v)
                for jt in range(2):
                    nc.tensor.matmul(
                        av_p[:, p * 2 * B: p * 2 * B + B],
                        va_t[:, s * W + (h * 2 + jt) * 33: s * W + (h * 2 + jt + 1) * 33],
                        es_t[:, p * 2 * B + jt * B: p * 2 * B + (jt + 1) * B],
                        start=(jt == 0), stop=(jt == 1),
                    ).then_inc(pe, 1)

            for b in range(NBLK):
                s = b % 2
                tensor.wait_ge(dmain, 16 * 3 * (b + 1))
                for h in range(H):
                    p = h % 2
                    v = 8 * b + h - 1
                    if v > 0:
                        tensor.wait_ge(act, v)
                    for jt in range(2):
                        nc.tensor.matmul(
                            sc_p[:, p * 2 * B + jt * B: p * 2 * B + (jt + 1) * B],
                            ka_t[:36, s * HB + h * B + jt * 128: s * HB + h * B + (jt + 1) * 128],
                            qa_t[:36, s * HB + h * B: s * HB + (h + 1) * B],
                            start=True, stop=True,
                        ).then_inc(pe, 1)
                    if h >= 1:
                        emit_av(b, h - 1)
                emit_av(b, H - 1)

        @block.scalar
        def _(scalar):
            for b in range(NBLK):
                for h in range(H):
                    p = h % 2
                    scalar.wait_ge(pe, pe_after_scores(b, h))
                    nc.scalar.activation(
                        es_t[:, p * 2 * B:(p + 1) * 2 * B],
                        sc_p[:, p * 2 * B:(p + 1) * 2 * B],
                        Exp,
                    ).then_inc(act, 1)

        @block.vector
        def _(vector):
            for b in range(NBLK):
                s = b % 2
                for h in range(H):
                    p = h % 2
                    vector.wait_ge(pe, pe_after_av(b, h))
                    if b >= 2 and h == 0:
                        vector.wait_ge(dmaout, 16 * (b - 1))
                    nc.vector.tensor_copy(
                        avsb_t[:, s * HB + h * B: s * HB + (h + 1) * B],
                        av_p[:, p * 2 * B: p * 2 * B + B],
                    ).then_inc(dve, 1)
    return nc


def _layernorm(x, g, b, eps=1e-5):
    mu = x.mean(-1, keepdims=True)
    var = x.var(-1, keepdims=True)
    return (x - mu) / np.sqrt(var + eps) * g + b


def kernel(x, coords, wq, wk, wv, w_rpe_w, w_out, b_out,
           g1, be1, g2, be2, ff_w1, ff_b1, ff_w2, ff_b2):
    from concourse.bass_utils import run_bass_kernel_spmd

    x = np.asarray(x, np.float32)
    coords = np.asarray(coords, np.float32)
    n = x.shape[0]
    nb = n // B

    order = np.argsort(coords[:, 0], kind="stable")
    xs = x[order]
    p = coords[order, 1:].reshape(nb, B, 2)

    xn = _layernorm(xs, np.asarray(g1, np.float32), np.asarray(be1, np.float32))
    q = (xn @ np.asarray(wq, np.float32).T).reshape(nb, B, H, D)
    k = (xn @ np.asarray(wk, np.float32).T).reshape(nb, B, H, D)
    v = (xn @ np.asarray(wv, np.float32).T).reshape(nb, B, H, D)

    W = np.asarray(w_rpe_w, np.float32).reshape(H, D, 2, NUM_W_PER_DIST)
    w2 = (W ** 2).mean(axis=(1, 3))  # [H, 2]

    scale = 1.0 / np.sqrt(np.float32(D))
    # qa[b, :, h*B+i]: rows 0-31 q^T*scale, 32: p0, 33: p1, 34: 1, 35: 1
    qa = np.empty((nb, 36, H * B), np.float32)
    ka = np.empty((nb, 36, H * B), np.float32)
    for h in range(H):
        sl = slice(h * B, (h + 1) * B)
        qa[:, :D, sl] = q[:, :, h, :].transpose(0, 2, 1) * scale
        qa[:, D + 0, sl] = p[:, :, 0]
        qa[:, D + 1, sl] = p[:, :, 1]
        qa[:, D + 2, sl] = 1.0
        qa[:, D + 3, sl] = 1.0
        ka[:, :D, sl] = k[:, :, h, :].transpose(0, 2, 1)
        ka[:, D + 0, sl] = 2.0 * w2[h, 0] * p[:, :, 0]
        ka[:, D + 1, sl] = 2.0 * w2[h, 1] * p[:, :, 1]
        ka[:, D + 2, sl] = -w2[h, 0] * p[:, :, 0] ** 2
        ka[:, D + 3, sl] = -w2[h, 1] * p[:, :, 1] ** 2

    # va[b, p128, (h*2+jt)*33 + c]: c<32 -> v[b, jt*128+p, h, c]; c==32 -> 1
    va = np.empty((nb, 128, H * 2 * 33), np.float32)
    for h in range(H):
        for jt in range(2):
            c0 = (h * 2 + jt) * 33
            va[:, :, c0:c0 + D] = v[:, jt * 128:(jt + 1) * 128, h, :]
            va[:, :, c0 + D] = 1.0

    qa = qa.astype(BF16)
    ka = ka.astype(BF16)
    va = va.astype(BF16)

    if "nc" not in _CACHE:
        _CACHE["nc"] = _build_nc()
    nc = _CACHE["nc"]

    nbc = NB_PER_CORE
    in_maps = [
        {"qa": qa[c * nbc:(c + 1) * nbc],
         "ka": ka[c * nbc:(c + 1) * nbc],
         "va": va[c * nbc:(c + 1) * nbc]}
        for c in range(NCORES)
    ]
    import time as _time
    _t0 = _time.time()
    res = run_bass_kernel_spmd(nc, in_maps, list(range(NCORES)))
    _CACHE["spmd_time_ns"] = int((_time.time() - _t0) * 1e9)
    outs = res.results if hasattr(res, "results") else res
    onum = np.concatenate([np.asarray(o["onum"], np.float32) for o in outs], axis=0)

    # onum [nb, 33, H*B] -> attention out [nb, B, H, D]
    onum = onum.reshape(nb, 33, H, B)
    num = onum[:, :D]                # [nb, D, H, B]
    den = onum[:, D]                 # [nb, H, B]
    att = (num / den[:, None]).transpose(0, 3, 2, 1)  # [nb, B, H, D]
    out_sorted = att.reshape(n, H * D)

    aggr = out_sorted @ np.asarray(w_out, np.float32).T + np.asarray(b_out, np.float32)
    xr = xs + aggr
    h1 = np.maximum(
        _layernorm(xr, np.asarray(g2, np.float32), np.asarray(be2, np.float32))
        @ np.asarray(ff_w1, np.float32).T + np.asarray(ff_b1, np.float32), 0.0)
    ff = h1 @ np.asarray(ff_w2, np.float32).T + np.asarray(ff_b2, np.float32)
    final_sorted = xr + ff

    result = np.empty_like(final_sorted)
    result[order] = final_sorted
    return result.astype(np.float32)



# revision 2
# speedup vs baseline: 1.8406x; 1.8406x over previous
"""Fused HEPT-style block attention + LN + FFN, fully on-device.

Host does: argsort by coords[:,0], gather, weight folding, scatter back.
Device does (per core, 32 blocks of 256 tokens): LN1, QKV projections,
per-head block attention with RPE bias, softmax, output projection, LN2,
FFN. Returns delta = aggr + ff (bf16); host adds the f32 x residual.
"""
import sys, os
for _p in ("/opt/trn_rl_repo", "/root/.axon_site/_ro/trn_rl_repo"):
    if os.path.isdir(_p) and _p not in sys.path:
        sys.path.insert(0, _p)
import numpy as np
import ml_dtypes

BF16 = ml_dtypes.bfloat16

NUM_HEADS = 8
HEAD_DIM = 32
NUM_W_PER_DIST = 8
BLOCK_SIZE = 256
N = 65536
NCORES = 8
B = BLOCK_SIZE
H = NUM_HEADS
D = HEAD_DIM
NB_PER_CORE = (N // B) // NCORES   # 32
NTOK = NB_PER_CORE * B             # 8192 tokens per core

_CACHE = {}


def _split_multiwaits(bir_bytes: bytes) -> bytes:
    """walrus in this container rejects >1 sync wait per instruction; hoist
    extras onto standalone EventSemaphore carriers placed just before."""
    import orjson
    j = orjson.loads(bir_bytes)
    n_new = 0
    for fn in j["functions"]:
        for bb in fn["blocks"]:
            out = []
            for ins in bb["instructions"]:
                si = ins.get("sync_info")
                waits = (si or {}).get("on_wait") or []
                if len(waits) > 1:
                    for w in waits[:-1]:
                        out.append({
                            "debug": ins.get("debug", 0),
                            "engine": ins["engine"],
                            "ins": [],
                            "name": f"wsplit-{n_new}",
                            "opcode": "EventSemaphore",
                            "outs": [],
                            "sync_info": {"on_update": [], "on_wait": [w]},
                        })
                        n_new += 1
                    si["on_wait"] = [waits[-1]]
                out.append(ins)
            bb["instructions"] = out
    return orjson.dumps(j)


def _build_nc(nblk=NB_PER_CORE):
    import concourse.bass as bass
    import concourse.mybir as mybir
    import concourse.tile as tile
    from concourse.masks import make_identity

    nc = bass.Bass()
    bf = mybir.dt.bfloat16
    f32 = mybir.dt.float32
    Alu = mybir.AluOpType
    Act = mybir.ActivationFunctionType
    ntok = nblk * B

    xd = nc.declare_dram_parameter("xd", [ntok, D], bf, isOutput=False)
    pd = nc.declare_dram_parameter("pd", [4, ntok], bf, isOutput=False)
    wqT_d = nc.declare_dram_parameter("wqT", [D, H * D], bf, isOutput=False)
    wkT_d = nc.declare_dram_parameter("wkT", [D, H * D], bf, isOutput=False)
    wvT_d = nc.declare_dram_parameter("wvT", [D, H * D], bf, isOutput=False)
    qkb_d = nc.declare_dram_parameter("qkb", [D, 2 * H], f32, isOutput=False)
    mc_d = nc.declare_dram_parameter("mc", [4, H], f32, isOutput=False)
    wo_d = nc.declare_dram_parameter("wo", [D, H * D], bf, isOutput=False)
    ffw_d = nc.declare_dram_parameter("ffw", [D, 2 * D], bf, isOutput=False)
    cvec_d = nc.declare_dram_parameter("cvec", [D, 3], f32, isOutput=False)
    od = nc.declare_dram_parameter("od", [D, ntok], bf, isOutput=True)

    with tile.TileContext(nc) as tc:
        with (
            tc.tile_pool(name="consts", bufs=1) as consts,
            tc.tile_pool(name="io", bufs=3) as io,
            tc.tile_pool(name="work", bufs=2) as work,
            tc.tile_pool(name="heads", bufs=3) as heads,
            tc.tile_pool(name="stats", bufs=3) as stats,
            tc.tile_pool(name="ps_sc", bufs=2, space="PSUM") as ps_sc,
            tc.tile_pool(name="ps_av", bufs=2, space="PSUM") as ps_av,
            tc.tile_pool(name="ps_proj", bufs=2, space="PSUM") as ps_proj,
            tc.tile_pool(name="ps_sm", bufs=2, space="PSUM") as ps_sm,
        ):
            # ---- constants ----
            ident = consts.tile([128, 128], bf)
            make_identity(nc, ident)
            eps_col = consts.tile([128, 1], f32)
            nc.vector.memset(eps_col, 1e-5)
            ones_c = consts.tile([128, 1], bf)       # den matmul lhsT
            nc.vector.memset(ones_c, 1.0)
            ones_r32 = consts.tile([1, D], f32)      # recip bcast lhsT
            nc.vector.memset(ones_r32, 1.0)
            ones2 = consts.tile([2, B], bf)          # qx ones rows source
            nc.vector.memset(ones2, 1.0)
            wqT = consts.tile([D, H * D], bf)
            nc.sync.dma_start(out=wqT, in_=wqT_d[:, :])
            wkT = consts.tile([D, H * D], bf)
            nc.sync.dma_start(out=wkT, in_=wkT_d[:, :])
            wvT = consts.tile([D, H * D], bf)
            nc.sync.dma_start(out=wvT, in_=wvT_d[:, :])
            qkb = consts.tile([D, 2 * H], f32)
            nc.sync.dma_start(out=qkb, in_=qkb_d[:, :])
            mc = consts.tile([4, H], f32)
            nc.sync.dma_start(out=mc, in_=mc_d[:, :])
            wo = consts.tile([D, H * D], bf)
            nc.sync.dma_start(out=wo, in_=wo_d[:, :])
            ffw = consts.tile([D, 2 * D], bf)
            nc.sync.dma_start(out=ffw, in_=ffw_d[:, :])
            cvec = consts.tile([D, 3], f32)
            nc.sync.dma_start(out=cvec, in_=cvec_d[:, :])

            xd_v = xd.rearrange("(nb c p) f -> nb p c f", c=2, p=128)

            for b in range(nblk):
                # ---- loads ----
                x_tok = io.tile([128, 2, D], bf)
                nc.sync.dma_start(out=x_tok, in_=xd_v[b])
                pr = io.tile([4, B], bf)
                nc.sync.dma_start(out=pr, in_=pd[:, b * B:(b + 1) * B])
                qx = io.tile([4, B], bf)
                nc.sync.dma_start(out=qx[0:2, :], in_=pd[0:2, b * B:(b + 1) * B])
                nc.sync.dma_start(out=qx[2:4, :], in_=ones2)

                # ---- LN1 (token-major) + transpose to feature-major ----
                z_tok = work.tile([128, 2, D], bf)
                zT_ps = ps_sm.tile([D, B], bf, tag="small")
                for c in range(2):
                    st = stats.tile([128, 6], f32)
                    nc.vector.bn_stats(out=st, in_=x_tok[:, c, :])
                    mv = stats.tile([128, 2], f32)
                    nc.vector.bn_aggr(out=mv, in_=st)
                    rstd = stats.tile([128, 1], f32)
                    nc.scalar.activation(out=rstd, in_=mv[:, 1:2],
                                         func=Act.Sqrt, bias=eps_col, scale=1.0)
                    nc.vector.reciprocal(out=rstd, in_=rstd)
                    nc.vector.tensor_scalar(out=z_tok[:, c, :], in0=x_tok[:, c, :],
                                            scalar1=mv[:, 0:1], scalar2=rstd,
                                            op0=Alu.subtract, op1=Alu.mult)
                    nc.tensor.transpose(zT_ps[:, c * 128:(c + 1) * 128],
                                        z_tok[:, c, :], ident)
                znT = work.tile([D, B], bf)
                nc.vector.tensor_copy(out=znT, in_=zT_ps)

                # ---- Q, K (feature-major), V (token-major) ----
                q_ps = ps_proj.tile([128, 2 * B], f32, tag="proj")
                k_ps = ps_proj.tile([128, 2 * B], f32, tag="proj")
                for s in range(2):
                    nc.tensor.matmul(q_ps[:, s * B:(s + 1) * B],
                                     wqT[:, s * 128:(s + 1) * 128], znT,
                                     start=True, stop=True)
                    nc.tensor.matmul(k_ps[:, s * B:(s + 1) * B],
                                     wkT[:, s * 128:(s + 1) * 128], znT,
                                     start=True, stop=True)
                q_sb = work.tile([D, H * B], bf)
                k_sb = work.tile([D, H * B], bf)
                for h in range(H):
                    s, hh = h // 4, h % 4
                    nc.vector.tensor_scalar_add(
                        out=q_sb[:, h * B:(h + 1) * B],
                        in0=q_ps[hh * D:(hh + 1) * D, s * B:(s + 1) * B],
                        scalar1=qkb[:, h:h + 1])
                    nc.vector.tensor_scalar_add(
                        out=k_sb[:, h * B:(h + 1) * B],
                        in0=k_ps[hh * D:(hh + 1) * D, s * B:(s + 1) * B],
                        scalar1=qkb[:, H + h:H + h + 1])
                v_ps = ps_proj.tile([128, 2 * B], f32, tag="proj")
                for jt in range(2):
                    nc.tensor.matmul(v_ps[:, jt * B:(jt + 1) * B],
                                     znT[:, jt * 128:(jt + 1) * 128], wvT,
                                     start=True, stop=True)
                vsb = work.tile([128, 2 * B], bf)
                nc.vector.tensor_copy(out=vsb, in_=v_ps)

                # ---- RPE bias rows per head from pr = [p0;p1;p0^2;p1^2] ----
                kx = work.tile([4, H * B], bf)
                for h in range(H):
                    nc.vector.tensor_scalar_mul(out=kx[:, h * B:(h + 1) * B],
                                                in0=pr, scalar1=mc[:, h:h + 1])

                # ---- per-head attention ----
                attn = work.tile([D, H * B], bf)
                for h in range(H):
                    sc = ps_sc.tile([128, 2 * B], f32, tag="sc")
                    for jt in range(2):
                        nc.tensor.matmul(
                            sc[:, jt * B:(jt + 1) * B],
                            k_sb[:, h * B + jt * 128:h * B + (jt + 1) * 128],
                            q_sb[:, h * B:(h + 1) * B],
                            start=True, stop=False)
                        nc.tensor.matmul(
                            sc[:, jt * B:(jt + 1) * B],
                            kx[:, h * B + jt * 128:h * B + (jt + 1) * 128],
                            qx, start=False, stop=True)
                    es = heads.tile([128, 2 * B], bf)
                    nc.scalar.activation(out=es, in_=sc, func=Act.Exp)
                    av = ps_av.tile([D + 1, B], f32, tag="av")
                    for jt in range(2):
                        nc.tensor.matmul(av[0:D, :],
                                         vsb[:, jt * B + h * D:jt * B + (h + 1) * D],
                                         es[:, jt * B:(jt + 1) * B],
                                         start=(jt == 0), stop=(jt == 1))
                    for jt in range(2):
                        nc.tensor.matmul(av[D:D + 1, :], ones_c,
                                         es[:, jt * B:(jt + 1) * B],
                                         start=(jt == 0), stop=(jt == 1))
                    recip = stats.tile([1, B], f32)
                    nc.vector.reciprocal(out=recip, in_=av[D:D + 1, :])
                    rb_ps = ps_sm.tile([D, B], f32, tag="small")
                    nc.tensor.matmul(rb_ps, ones_r32, recip, start=True, stop=True)
                    rb_sb = heads.tile([D, B], f32, tag="rb")
                    nc.scalar.activation(out=rb_sb, in_=rb_ps, func=Act.Copy)
                    nc.vector.tensor_mul(out=attn[:, h * B:(h + 1) * B],
                                         in0=av[0:D, :], in1=rb_sb)

                # ---- output projection (+ b_out incl. folded vb) ----
                agg_ps = ps_sm.tile([D, B], f32, tag="small")
                for h in range(H):
                    nc.tensor.matmul(agg_ps, wo[:, h * D:(h + 1) * D],
                                     attn[:, h * B:(h + 1) * B],
                                     start=(h == 0), stop=(h == H - 1))
                aggr = work.tile([D, B], bf)
                nc.vector.tensor_scalar_add(out=aggr, in0=agg_ps,
                                            scalar1=cvec[:, 2:3])

                # ---- xr = x + aggr (token-major), LN2, transpose ----
                xr = work.tile([128, 2, D], bf)
                z2 = work.tile([128, 2, D], bf)
                z2T_ps = ps_sm.tile([D, B], bf, tag="small")
                for c in range(2):
                    agT_ps = ps_sm.tile([128, D], bf, tag="small")
                    nc.tensor.transpose(agT_ps, aggr[:, c * 128:(c + 1) * 128],
                                        ident[0:D, 0:D])
                    nc.vector.tensor_add(out=xr[:, c, :], in0=x_tok[:, c, :],
                                         in1=agT_ps)
                    st2 = stats.tile([128, 6], f32)
                    nc.vector.bn_stats(out=st2, in_=xr[:, c, :])
                    mv2 = stats.tile([128, 2], f32)
                    nc.vector.bn_aggr(out=mv2, in_=st2)
                    rstd2 = stats.tile([128, 1], f32)
                    nc.scalar.activation(out=rstd2, in_=mv2[:, 1:2],
                                         func=Act.Sqrt, bias=eps_col, scale=1.0)
                    nc.vector.reciprocal(out=rstd2, in_=rstd2)
                    nc.vector.tensor_scalar(out=z2[:, c, :], in0=xr[:, c, :],
                                            scalar1=mv2[:, 0:1], scalar2=rstd2,
                                            op0=Alu.subtract, op1=Alu.mult)
                    nc.tensor.transpose(z2T_ps[:, c * 128:(c + 1) * 128],
                                        z2[:, c, :], ident)
                z2T = work.tile([D, B], bf)
                nc.vector.tensor_copy(out=z2T, in_=z2T_ps)

                # ---- FFN + delta out ----
                f1_ps = ps_sm.tile([D, B], f32, tag="small")
                nc.tensor.matmul(f1_ps, ffw[:, 0:D], z2T, start=True, stop=True)
                h1 = work.tile([D, B], bf)
                nc.scalar.activation(out=h1, in_=f1_ps, func=Act.Relu,
                                     bias=cvec[:, 0:1], scale=1.0)
                f2_ps = ps_sm.tile([D, B], f32, tag="small")
                nc.tensor.matmul(f2_ps, ffw[:, D:2 * D], h1, start=True, stop=True)
                dtmp = work.tile([D, B], f32)
                nc.vector.tensor_scalar_add(out=dtmp, in0=f2_ps,
                                            scalar1=cvec[:, 1:2])
                delta = io.tile([D, B], bf)
                nc.vector.tensor_add(out=delta, in0=dtmp, in1=aggr)
                nc.sync.dma_start(out=od[:, b * B:(b + 1) * B], in_=delta)

    nc.finalize()
    _orig = type(nc).to_json_bytes
    _json_cache = []
    def _cached_json():
        if not _json_cache:
            _json_cache.append(_split_multiwaits(_orig(nc)))
        return _json_cache[0]
    nc.to_json_bytes = _cached_json
    return nc


def _fold_weights(wq, wk, wv, w_rpe_w, w_out, b_out, g1, be1, g2, be2,
                  ff_w1, ff_b1, ff_w2, ff_b2):
    """Host-side weight folding. Returns dict of device weight arrays."""
    f = np.float32
    wq, wk, wv = np.asarray(wq, f), np.asarray(wk, f), np.asarray(wv, f)
    g1, be1 = np.asarray(g1, f), np.asarray(be1, f)
    g2, be2 = np.asarray(g2, f), np.asarray(be2, f)
    w_out, b_out = np.asarray(w_out, f), np.asarray(b_out, f)
    ff_w1, ff_b1 = np.asarray(ff_w1, f), np.asarray(ff_b1, f)
    ff_w2, ff_b2 = np.asarray(ff_w2, f), np.asarray(ff_b2, f)
    scale = f(1.0 / np.sqrt(f(D)))

    wq_g = wq * g1[None, :] * scale          # [256, 32]
    wk_g = wk * g1[None, :]
    wv_g = wv * g1[None, :]
    qb = (be1 @ wq.T) * scale                # [256]
    kb = be1 @ wk.T
    vb = be1 @ wv.T
    bo = b_out + vb @ w_out.T                # [32] (vb folds: softmax rows sum to 1)

    W = np.asarray(w_rpe_w, f).reshape(H, D, 2, NUM_W_PER_DIST)
    w2 = (W ** 2).mean(axis=(1, 3))          # [H, 2]
    mc = np.stack([2.0 * w2[:, 0], 2.0 * w2[:, 1], -w2[:, 0], -w2[:, 1]],
                  axis=0)                    # [4, H]

    ff1g = ff_w1 * g2[None, :]               # [32, 32]
    fb1 = be2 @ ff_w1.T + ff_b1              # [32]

    qkb = np.concatenate([qb.reshape(H, D).T, kb.reshape(H, D).T],
                         axis=1)             # [32, 16]
    wo_sb = w_out.T.reshape(H, D, D).transpose(1, 0, 2).reshape(D, H * D)
    ffw = np.concatenate([ff1g.T, ff_w2.T], axis=1)                   # [32, 64]
    cvec = np.stack([fb1, ff_b2, bo], axis=1)                         # [32, 3]

    return {
        "wqT": wq_g.T.astype(BF16).copy(),   # [32, 256]
        "wkT": wk_g.T.astype(BF16).copy(),
        "wvT": wv_g.T.astype(BF16).copy(),
        "qkb": np.ascontiguousarray(qkb, f),
        "mc": np.ascontiguousarray(mc, f),   # [4, 8]
        "wo": wo_sb.astype(BF16).copy(),
        "ffw": ffw.astype(BF16).copy(),
        "cvec": np.ascontiguousarray(cvec, f),
    }


def _make_runner(nc):
    """Persistent-jit variant of run_bass_kernel_spmd's axon path: identical
    _bass_exec custom-call execution on cores 0-7, but the compiled+loaded
    executable is cached across calls (a fresh jax.jit per call would reload
    the NEFF onto the devices every time, ~0.5s)."""
    import jax
    import numpy as _np
    from jax.sharding import Mesh, PartitionSpec
    from jax.experimental.shard_map import shard_map
    import concourse.mybir as mybir
    from concourse.bass2jax import (install_neuronx_cc_hook, _bass_exec_p,
                                    partition_id_tensor)

    install_neuronx_cc_hook()
    partition_name = nc.partition_id_tensor.name if nc.partition_id_tensor else None
    in_names, out_names, out_avals = [], [], []
    for alloc in nc.m.functions[0].allocations:
        if not isinstance(alloc, mybir.MemoryLocationSet):
            continue
        name = alloc.memorylocations[0].name
        if alloc.kind == "ExternalInput":
            if name != partition_name:
                in_names.append(name)
        elif alloc.kind == "ExternalOutput":
            out_names.append(name)
            out_avals.append(jax.core.ShapedArray(
                tuple(alloc.tensor_shape), mybir.dt.np(alloc.dtype)))
    n_params = len(in_names)
    n_outs = len(out_avals)
    all_names = in_names + out_names
    if partition_name is not None:
        all_names.append(partition_name)
    donate = tuple(range(n_params, n_params + n_outs))

    def _body(*args):
        operands = list(args)
        if partition_name is not None:
            operands.append(partition_id_tensor())
        return tuple(_bass_exec_p.bind(
            *operands, out_avals=tuple(out_avals), in_names=tuple(all_names),
            out_names=tuple(out_names), lowering_input_output_aliases=(),
            sim_require_finite=True, sim_require_nnan=True, nc=nc))

    devices = jax.devices()[:NCORES]
    mesh = Mesh(_np.asarray(devices), ("core",))
    sharded = jax.jit(
        shard_map(_body, mesh=mesh,
                  in_specs=(PartitionSpec("core"),) * (n_params + n_outs),
                  out_specs=(PartitionSpec("core"),) * n_outs,
                  check_rep=False),
        donate_argnums=donate, keep_unused=True)

    def run(in_maps):
        concat_in = [
            _np.concatenate([_np.asarray(m[nm]) for m in in_maps], axis=0)
            for nm in in_names]
        concat_zeros = [
            _np.zeros((NCORES * a.shape[0], *a.shape[1:]), a.dtype)
            for a in out_avals]
        out_arrs = sharded(*concat_in, *concat_zeros)
        return [
            {nm: _np.asarray(out_arrs[i]).reshape(NCORES, *out_avals[i].shape)[c]
             for i, nm in enumerate(out_names)}
            for c in range(NCORES)]

    return run


def kernel(x, coords, wq, wk, wv, w_rpe_w, w_out, b_out,
           g1, be1, g2, be2, ff_w1, ff_b1, ff_w2, ff_b2):
    import time as _time

    x = np.asarray(x, np.float32)
    coords = np.asarray(coords, np.float32)
    n = x.shape[0]

    order = np.argsort(coords[:, 0], kind="stable")
    xs = x[order]
    p = coords[order, 1:]                                # [N, 2]
    p4 = np.stack([p[:, 0], p[:, 1], p[:, 0] ** 2, p[:, 1] ** 2])  # [4, N]

    wts = _fold_weights(wq, wk, wv, w_rpe_w, w_out, b_out, g1, be1,
                        g2, be2, ff_w1, ff_b1, ff_w2, ff_b2)
    xs_bf = xs.astype(BF16)
    p4_bf = p4.astype(BF16)

    if "nc" not in _CACHE:
        _CACHE["nc"] = _build_nc()
        _CACHE["runner"] = _make_runner(_CACHE["nc"])
    runner = _CACHE["runner"]

    in_maps = []
    for c in range(NCORES):
        m = {"xd": xs_bf[c * NTOK:(c + 1) * NTOK],
             "pd": np.ascontiguousarray(p4_bf[:, c * NTOK:(c + 1) * NTOK])}
        m.update(wts)
        in_maps.append(m)

    _t0 = _time.time()
    outs = runner(in_maps)
    _CACHE["spmd_time_ns"] = int((_time.time() - _t0) * 1e9)

    # od per core: [32, NTOK] bf16 feature-major delta
    delta = np.concatenate(
        [np.asarray(o["od"], np.float32).T for o in outs], axis=0)  # [N, 32]
    final_sorted = xs + delta
    result = np.empty_like(final_sorted)
    result[order] = final_sorted
    return result.astype(np.float32)
